# revision 1
# baseline (speedup 1.0000x reference)
"""ChannelCovarianceBlock Trainium2 kernel.

Computes, for queries x1 (B, C, h, w) and support sets x2 (nw, Bs, C, h, w):
  cov_n = Cov(x2[n].reshape(Bs*C, hw))            (hw, hw) per class
  d     = normalize-and-center rows of x1.reshape(B*C, hw)
  sim[b, n, c] = d[bc] @ cov_n @ d[bc]^T          -> (B, nw*C)

Sharding: data-parallel over B across 8 NeuronCores (32 queries each);
each core computes all 10 class covariances from the full x2 (redundant
but collective-free) using the Gram identity cov = (X^T X - s s^T/N)/(N-1).

Per-core dataflow:
  stage 0: preprocess queries in place (SBUF-resident D), build D^T via
           PE transposes, spill D^T to DRAM.
  stage 1 (per class): Gram matmuls + rank-1 mean correction -> cov_n SBUF.
  stage 2 (per class, m-tile): S = D @ cov_n on PE (float32r), then a
           fused multiply+row-reduce (scalar_tensor_tensor with accum_out;
           tensor_tensor_reduce crashes trn2 HW) against the resident D
           gives sim[:, n].

Measured (8 cores, steady state): ~1.6-1.9 ms/exec (device-state
dependent), rel err 4.5e-05.
"""

import os
import sys

for _p in ("/opt/trn_rl_repo", "/root/.axon_site/_ro/trn_rl_repo"):
    if os.path.isdir(_p) and _p not in sys.path:
        sys.path.append(_p)

import numpy as np

# Problem constants (hardcoded per spec).
B, C, H, W = 256, 128, 28, 28
NW, BS = 10, 10
HW = H * W            # 784
N_CORES = 8
BSH = B // N_CORES    # 32 queries per core
NI = BSH * C          # 4096 rows per core
NR = BS * C           # 1280 support rows per class

# K-tiles over the hw contraction dim (partition dim <= 128).
KT = [(k * 128, min(128, HW - k * 128)) for k in range((HW + 127) // 128)]
NKT = len(KT)         # 7
# N-tiles over the hw free dim (>=256 keeps float32r at 1 cycle/row).
QT = [(0, 392), (392, 392)]
MT = NI // 128        # 32 i-tiles per core

_STATE = {}


def _build_program(mm_dtype_name="float32r", stages=None, repeat=None):
    if stages is None:
        stages = os.environ.get("CCB_STAGES", "full")
    if repeat is None:
        repeat = int(os.environ.get("CCB_REPEAT", "1"))
    import concourse.bass as bass
    import concourse.bacc as bacc
    import concourse.tile as tile
    from concourse import mybir
    from concourse.masks import make_identity
    from contextlib import ExitStack

    f32 = mybir.dt.float32
    # Matmul operand dtype: float32r runs the PE at 1 cycle/row (vs 4 for
    # fp32) for N>=256. All f32r-consumed tiles must be f32r-typed with
    # walrus-approved producers (DMA from f32r DRAM, or DVE/ACT rounding
    # copies) -- the BIR verifier enforces this.
    mmdt = getattr(mybir.dt, mm_dtype_name)

    nc = bacc.Bacc()
    x1s = nc.declare_dram_parameter("x1s", [NI, HW], f32, isOutput=False)
    x2d = nc.declare_dram_parameter("x2", [NW, NR, HW], mmdt, isOutput=False)
    out = nc.declare_dram_parameter("out", [NI, NW], f32, isOutput=True)

    AF = mybir.ActivationFunctionType
    OP = mybir.AluOpType

    with tile.TileContext(nc) as tc:
        with ExitStack() as ctx:
            persist = ctx.enter_context(tc.tile_pool(name="persist", bufs=1))
            ident = persist.tile([128, 128], f32, tag="ident")
            make_identity(nc, ident)
            ones_f = persist.tile([128, 1], f32, tag="ones_f")
            nc.vector.memset(ones_f, 1.0)
            ones = persist.tile([128, 1], mmdt, tag="ones")
            nc.vector.tensor_copy(out=ones, in_=ones_f)
            # D stays resident: d_res[:, m, q] = D[m*128 + p, q]
            d_res = persist.tile([128, MT, HW], f32, tag="d_res")
            out_acc = persist.tile([128, MT, NW], f32, tag="out_acc")
            if stages != "full":
                nc.vector.memset(out_acc, 0.0)

            dram = ctx.enter_context(tc.tile_pool(name="dram", bufs=1, space="DRAM"))
            # dtT_dram[m][p, kt, i] = D[m*128 + i, kt*128 + p] (full k-blocks)
            dtT_dram = dram.tile([MT, 128, NKT - 1, 128], mmdt, tag="dtT")
            # remainder k-block (16 rows of p)
            dtr_dram = dram.tile([MT, KT[-1][1], 128], mmdt, tag="dtr")

            scr_pool = ctx.enter_context(tc.tile_pool(name="scr", bufs=2))
            stats = ctx.enter_context(tc.tile_pool(name="stats", bufs=4))

            # Optional on-device repeat loop (timing amplification only).
            if repeat > 1:
                ctx.enter_context(tc.For_i(0, repeat, 1))

            # ---- Stage 0: query preprocessing + D^T build ----
            with tc.tile_pool(name="psum_t", bufs=2, space="PSUM") as psum_t, \
                 tc.tile_pool(name="dtw", bufs=2) as dtw_pool:
                for m in range(MT):
                    rows = slice(m * 128, (m + 1) * 128)
                    dsl = d_res[:, m, :]
                    nc.sync.dma_start(out=dsl, in_=x1s[rows, :])
                    sq = scr_pool.tile([128, HW], f32, tag="scr")
                    sumsq = stats.tile([128, 1], f32, tag="sumsq")
                    # ACT: sq = x^2 (discarded), sumsq = row-sum(x^2)
                    nc.scalar.activation(
                        out=sq, in_=dsl, func=AF.Square, accum_out=sumsq
                    )
                    s1 = stats.tile([128, 1], f32, tag="s1")
                    nc.vector.tensor_reduce(
                        out=s1, in_=dsl, axis=mybir.AxisListType.X, op=OP.add
                    )
                    nrm = stats.tile([128, 1], f32, tag="nrm")
                    nc.scalar.activation(out=nrm, in_=sumsq, func=AF.Sqrt)
                    rn = stats.tile([128, 1], f32, tag="rn")
                    nc.vector.reciprocal(out=rn, in_=nrm)
                    ms = stats.tile([128, 1], f32, tag="ms")
                    nc.scalar.mul(out=ms, in_=s1, mul=1.0 / HW)
                    # d = (x - mean) * (1/||x||), in place
                    nc.vector.tensor_scalar(
                        out=dsl, in0=dsl, scalar1=ms, scalar2=rn,
                        op0=OP.subtract, op1=OP.mult,
                    )
                    dtw = dtw_pool.tile([128, NKT - 1, 128], mmdt, tag="dtw")
                    dtr = dtw_pool.tile([KT[-1][1], 128], mmdt, tag="dtr")
                    for kt, (koff, klen) in enumerate(KT):
                        pt = psum_t.tile([128, 128], f32, tag="pt")
                        nc.tensor.transpose(
                            out=pt[:klen, :128],
                            in_=dsl[:, koff:koff + klen],
                            identity=ident,
                        )
                        dst = dtw[:, kt, :] if kt < NKT - 1 else dtr
                        nc.vector.tensor_copy(out=dst, in_=pt[:klen, :128])
                    nc.sync.dma_start(out=dtT_dram[m], in_=dtw)
                    nc.sync.dma_start(out=dtr_dram[m], in_=dtr)

            # ---- Stages 1+2 per class ----
            xs_pool = ctx.enter_context(tc.tile_pool(name="xsup", bufs=1))
            cov_pool = ctx.enter_context(tc.tile_pool(name="cov", bufs=2))
            row_pool = ctx.enter_context(tc.tile_pool(name="rows", bufs=1))
            dts_pool = ctx.enter_context(tc.tile_pool(name="dts", bufs=4))
            psum_s = ctx.enter_context(
                tc.tile_pool(name="psum_s", bufs=4, space="PSUM")
            )
            psum_m = ctx.enter_context(
                tc.tile_pool(name="psum_mean", bufs=1, space="PSUM")
            )

            RTN = NR // 128  # 10 row-tiles per class
            for n in range(NW if stages != "0" else 0):
                xs = xs_pool.tile([128, RTN, HW], mmdt, tag="xs")
                for rt in range(RTN):
                    nc.sync.dma_start(
                        out=xs[:, rt, :], in_=x2d[n, rt * 128:(rt + 1) * 128, :]
                    )
                # column sums s (1, HW) via ones-matmul; psum sub-tiles are
                # bank-aligned (512-elem stride) so no matmul crosses a bank.
                pm = psum_m.tile([1, len(QT), 512], f32, tag="pm")
                for rt in range(RTN):
                    for qi, (qoff, qlen) in enumerate(QT):
                        nc.tensor.matmul(
                            pm[:1, qi, :qlen],
                            lhsT=ones[:, :1],
                            rhs=xs[:, rt, qoff:qoff + qlen],
                            start=(rt == 0),
                            stop=(rt == RTN - 1),
                        )
                srow = row_pool.tile([1, HW], mmdt, tag="srow")
                ssrow = row_pool.tile([1, HW], mmdt, tag="ssrow")
                for qi, (qoff, qlen) in enumerate(QT):
                    qs = slice(qoff, qoff + qlen)
                    nc.scalar.mul(out=srow[:, qs], in_=pm[:1, qi, :qlen], mul=1.0)
                    nc.scalar.mul(out=ssrow[:, qs], in_=pm[:1, qi, :qlen], mul=-1.0 / NR)

                # cov_n = (X^T X - s s^T / NR) / (NR - 1), tiled (p-block, q)
                cov = cov_pool.tile([128, NKT, HW], mmdt, tag="cov")
                for mc, (mcoff, mclen) in enumerate(KT):
                    for (qoff, qlen) in QT:
                        ps = psum_s.tile([128, 392], f32, tag="ps")
                        for rt in range(RTN):
                            nc.tensor.matmul(
                                ps[:mclen, :qlen],
                                lhsT=xs[:, rt, mcoff:mcoff + mclen],
                                rhs=xs[:, rt, qoff:qoff + qlen],
                                start=(rt == 0),
                                stop=False,
                            )
                        nc.tensor.matmul(
                            ps[:mclen, :qlen],
                            lhsT=ssrow[:1, mcoff:mcoff + mclen],
                            rhs=srow[:1, qoff:qoff + qlen],
                            start=False,
                            stop=True,
                        )
                        nc.scalar.mul(
                            out=cov[:mclen, mc, qoff:qoff + qlen],
                            in_=ps[:mclen, :qlen],
                            mul=1.0 / (NR - 1),
                        )

                # Stage 2: sim[:, n] = rowsum((D @ cov_n) * D) per m-tile
                for m in range(MT if stages not in ("0", "01") else 0):
                    dtw2 = dts_pool.tile([128, NKT - 1, 128], mmdt, tag="dts")
                    # 3-way split: parallel HWDGE queues for the hot D^T
                    # stream (1 big DMA = 1 queue = too slow; 6-way = too
                    # many dispatches). Measured optimum.
                    nc.sync.dma_start(out=dtw2[:, :2, :], in_=dtT_dram[m, :, :2, :])
                    nc.sync.dma_start(out=dtw2[:, 2:4, :], in_=dtT_dram[m, :, 2:4, :])
                    nc.sync.dma_start(out=dtw2[:, 4:, :], in_=dtT_dram[m, :, 4:, :])
                    dtr2 = dts_pool.tile([KT[-1][1], 128], mmdt, tag="dtsr")
                    nc.sync.dma_start(out=dtr2, in_=dtr_dram[m])
                    if stages == "2d":
                        scr = scr_pool.tile([128, HW], f32, tag="scr")
                        nc.vector.tensor_copy(out=scr[:, :128], in_=dtw2[:, 0, :])
                        continue
                    if stages == "2w":
                        # matmuls with weights from a fixed resident tile
                        # (no dependence on the streamed dtw2) - stall probe
                        for qi, (qoff, qlen) in enumerate(QT):
                            ps = psum_s.tile([128, 392], f32, name="ps", tag="ps")
                            for kt, (koff, klen) in enumerate(KT):
                                nc.tensor.matmul(
                                    ps[:128, :qlen],
                                    lhsT=xs[:klen, 0, :128],
                                    rhs=cov[:klen, kt, qoff:qoff + qlen],
                                    start=(kt == 0),
                                    stop=(kt == NKT - 1),
                                )
                            scr = scr_pool.tile([128, HW], f32, tag="scr")
                            nc.vector.tensor_copy(out=scr[:, :qlen], in_=ps[:, :qlen])
                        continue
                    acc = out_acc[:, m, n:n + 1]
                    pp = stats.tile([128, 2], f32, name="pp", tag="pp")
                    for qi, (qoff, qlen) in enumerate(QT):
                        ps = psum_s.tile([128, 392], f32, tag="ps")
                        for kt, (koff, klen) in enumerate(KT):
                            lhsT = dtw2[:, kt, :] if kt < NKT - 1 else dtr2
                            nc.tensor.matmul(
                                ps[:128, :qlen],
                                lhsT=lhsT,
                                rhs=cov[:klen, kt, qoff:qoff + qlen],
                                start=(kt == 0),
                                stop=(kt == NKT - 1),
                            )
                        scr = scr_pool.tile([128, HW], f32, tag="scr")
                        # out=(ps*1)*d elementwise; accum_out = row-sum
                        nc.vector.scalar_tensor_tensor(
                            out=scr[:, :qlen],
                            in0=ps[:, :qlen],
                            scalar=1.0,
                            in1=d_res[:, m, qoff:qoff + qlen],
                            op0=OP.mult,
                            op1=OP.mult,
                            accum_out=pp[:, qi:qi + 1],
                        )
                    nc.vector.tensor_reduce(
                        out=acc, in_=pp, axis=mybir.AxisListType.X, op=OP.add
                    )

            for m in range(MT):
                nc.sync.dma_start(
                    out=out[m * 128:(m + 1) * 128, :], in_=out_acc[:, m, :]
                )

    # Bacc defers register allocation to compile(); run_bass_via_pjrt
    # serializes the module as-is, so finalize here.
    nc.finalize()
    return nc


def get_program():
    key = "nc"
    if key not in _STATE:
        _STATE[key] = _build_program(
            os.environ.get("CCB_MM_DTYPE", "float32r")
        )
    return _STATE[key]


def make_in_maps(x1, x2):
    x1f = np.ascontiguousarray(
        np.asarray(x1, dtype=np.float32).reshape(B * C, HW)
    )
    x2f = np.ascontiguousarray(
        np.asarray(x2, dtype=np.float32).reshape(NW, NR, HW)
    )
    return [
        {"x1s": x1f[c * NI:(c + 1) * NI], "x2": x2f}
        for c in range(N_CORES)
    ]


def assemble_output(core_outs):
    # per-core (NI, NW) -> (BSH, NW*C); concat over cores -> (B, NW*C)
    parts = [
        o.reshape(BSH, C, NW).transpose(0, 2, 1).reshape(BSH, NW * C)
        for o in core_outs
    ]
    return np.ascontiguousarray(np.concatenate(parts, axis=0), dtype=np.float32)


def kernel(x1, x2):
    from concourse.bass_utils import run_bass_kernel_spmd

    nc = get_program()
    in_maps = make_in_maps(x1, x2)
    res = run_bass_kernel_spmd(nc, in_maps, list(range(N_CORES)))
    return assemble_output([res.results[i]["out"] for i in range(N_CORES)])



# revision 9
# speedup vs baseline: 1.9157x; 1.9157x over previous
"""ChannelCovarianceBlock Trainium2 kernel (fp8 DoubleRow version).

Computes, for queries x1 (B, C, h, w) and support sets x2 (nw, Bs, C, h, w):
  cov_n = Cov(x2[n].reshape(Bs*C, hw))            (hw, hw) per class
  d     = normalize-and-center rows of x1.reshape(B*C, hw)
  sim[b, n, c] = d[bc] @ cov_n @ d[bc]^T          -> (B, nw*C)

Sharding: data-parallel over B across 8 NeuronCores (32 queries each);
each core computes all 10 class covariances from the full x2 (redundant
but collective-free) using the Gram identity cov = (X^T X - s s^T/N)/(N-1).

Numerics: matmuls run in fp8e4 (e4m3) with MatmulPerfMode.DoubleRow
(0.5 PE cycles/row, 2x bf16 throughput). To survive fp8's 3-bit
mantissa, the covariance is split as cov = I + V: the exact base term
||d||^2 = 1 - hw*m^2 (m = row mean of the normalized query) is computed
from stage-0 stats in f32, and only the small-valued V = cov - I is
quantized to fp8 (the I subtraction happens inside PSUM via an exact
f32r matmul against a shifted-identity tile). d is scaled by 16 and V
by 16 before fp8 quantization; the 1/256 descale folds into the final
multiply-reduce. Measured rel err ~8e-3 (numpy model 7.8e-3).

Per-core dataflow (single pass, no DRAM spills):
  phase A (interleaved per m-tile / per class to keep PE busy):
    stage 0: load x1 rows (bf16), compute norm/mean stats, write the
             normalized d as bf16 (d_res, stt operand) and build D^T in
             fp8 via PE transposes (dtT, matmul lhsT), both SBUF-resident.
    gram:    per class, 5 DoubleRow Gram matmuls per (k-block, q-chunk)
             + rank-1 mean correction + f32r -(NR-1)*I matmul, then
             copy PSUM -> V fp8 (all 10 classes stay resident).
  phase B: per (class, m-tile): 8 matmuls (6 DoubleRow + 2 k=16
           remainder) -> S = D @ V in PSUM, then one DVE
           scalar_tensor_tensor (S * d_res, accum_out) -> out_acc.
           Finally out_acc += base, DMA out.
"""

import os
import sys

for _p in ("/opt/trn_rl_repo", "/root/.axon_site/_ro/trn_rl_repo"):
    if os.path.isdir(_p) and _p not in sys.path:
        sys.path.append(_p)

import numpy as np

# Problem constants (hardcoded per spec).
B, C, H, W = 256, 128, 28, 28
NW, BS = 10, 10
HW = H * W            # 784
N_CORES = 8
BSH = B // N_CORES    # 32 queries per core
NI = BSH * C          # 4096 rows per core
NR = BS * C           # 1280 support rows per class
RTN = NR // 128       # 10 row-tiles per class

# K-tiles over the hw contraction dim (partition dim <= 128).
KT = [(k * 128, min(128, HW - k * 128)) for k in range((HW + 127) // 128)]
NKT = len(KT)         # 7 (6 full + 16-row remainder)
NKT8 = 8              # k-tile slots incl. zero pad so kt (6,7) forms a DR pair
NDR = 3               # DoubleRow k-tile pairs (0,1)(2,3)(4,5); kt=6 plain
QT = [(0, 392), (392, 392)]
MT = NI // 128        # 32 i-tiles per core

SD = 16.0             # d scale before fp8 quantization
SC = 16.0             # V scale before fp8 quantization
EYE_OFF = 384         # identity block column offset in the EYE tile

_STATE = {}


def _build_program(repeat=None):
    if repeat is None:
        repeat = int(os.environ.get("CCB_REPEAT", "1"))
    import concourse.bass as bass
    import concourse.bacc as bacc
    import concourse.tile as tile
    from concourse import mybir
    from concourse.masks import make_identity
    from contextlib import ExitStack

    f32 = mybir.dt.float32
    f32r = mybir.dt.float32r
    bf16 = mybir.dt.bfloat16
    fp8 = mybir.dt.float8e4
    DRM = mybir.MatmulPerfMode.DoubleRow
    ALPHA = float(np.sqrt(NR - 1.0))

    nc = bacc.Bacc()
    x1s = nc.declare_dram_parameter("x1s", [NI, HW], bf16, isOutput=False)
    x2d = nc.declare_dram_parameter("x2", [NW, RTN, 128, HW], fp8, isOutput=False)
    out = nc.declare_dram_parameter("out", [MT, 128, NW], f32, isOutput=True)
    debug = os.environ.get("CCB_DEBUG") == "1"
    if debug:
        dbg_dtT = nc.declare_dram_parameter(
            "dbg_dtT", [128, MT, NKT8, 128], fp8, isOutput=True)
        dbg_cov = nc.declare_dram_parameter(
            "dbg_cov", [128, NW, NKT8, HW], fp8, isOutput=True)
        dbg_base = nc.declare_dram_parameter(
            "dbg_base", [128, MT], f32, isOutput=True)
        dbg_dres = nc.declare_dram_parameter(
            "dbg_dres", [128, MT, 2, 392], bf16, isOutput=True)

    AF = mybir.ActivationFunctionType
    OP = mybir.AluOpType

    with tile.TileContext(nc) as tc:
        with ExitStack() as ctx:
            persist = ctx.enter_context(tc.tile_pool(name="persist", bufs=1))
            ident_f = persist.tile([128, 128], f32, tag="ident_f")
            make_identity(nc, ident_f)
            # f32r copies must come from walrus-approved producers (DVE).
            ident = persist.tile([128, 128], f32r, tag="ident")
            nc.vector.tensor_copy(out=ident, in_=ident_f)
            # AI = +alpha*I, EYE carries -alpha at [p, EYE_OFF+p]; their
            # product in PSUM subtracts (NR-1)*I from the Gram exactly.
            ai = persist.tile([128, 128], f32r, tag="ai")
            nc.vector.tensor_scalar(
                out=ai, in0=ident_f, scalar1=ALPHA, scalar2=None, op0=OP.mult
            )
            eye_f = persist.tile([128, HW], f32, tag="eye_f")
            nc.vector.memset(eye_f, 0.0)
            nc.vector.tensor_scalar(
                out=eye_f[:, EYE_OFF:EYE_OFF + 128], in0=ident_f,
                scalar1=-ALPHA, scalar2=None, op0=OP.mult,
            )
            eye = persist.tile([128, HW], f32r, tag="eye")
            nc.vector.tensor_copy(out=eye, in_=eye_f)
            # DR weight APs need even, 16B-aligned outer free steps.
            ones2 = persist.tile([128, 2, 16], fp8, tag="ones2")
            nc.vector.memset(ones2, 1.0)
            # stt operand: normalized d, bf16, (2, 392) layout to match psum
            d_res = persist.tile([128, MT, 2, 392], bf16, tag="d_res")
            # matmul lhsT: D^T in fp8, scaled by SD
            dtT = persist.tile([128, MT, NKT8, 128], fp8, tag="dtT")
            # all 10 class V matrices, fp8, scaled by SC
            cov = persist.tile([128, NW, NKT8, HW], fp8, tag="cov")
            # zero the DR pad: kt=7 plane and partitions 16.. of kt=6
            nc.vector.memset(dtT[:, :, NKT8 - 1, :], 0.0)
            nc.vector.memset(dtT[:, :, NKT - 1, :], 0.0)
            nc.vector.memset(cov[:, :, NKT8 - 1, :], 0.0)
            nc.vector.memset(cov[:, :, NKT - 1, :], 0.0)
            out_acc = persist.tile([128, MT, NW], f32, tag="out_acc")
            base_t = persist.tile([128, MT], f32, tag="base")

            xw_pool = ctx.enter_context(tc.tile_pool(name="xw", bufs=3))
            dn_pool = ctx.enter_context(tc.tile_pool(name="dn", bufs=2))
            sq_pool = ctx.enter_context(tc.tile_pool(name="sq", bufs=2))
            stats = ctx.enter_context(tc.tile_pool(name="stats", bufs=6))
            xs_pool = ctx.enter_context(tc.tile_pool(name="xs", bufs=2))
            row_pool = ctx.enter_context(tc.tile_pool(name="rows", bufs=2))
            scr_pool = ctx.enter_context(tc.tile_pool(name="scr", bufs=2))

            ps_pool = ctx.enter_context(
                tc.tile_pool(name="ps", bufs=2, space="PSUM")
            )
            psg_pool = ctx.enter_context(
                tc.tile_pool(name="psg", bufs=2, space="PSUM")
            )
            pm_pool = ctx.enter_context(
                tc.tile_pool(name="pm", bufs=1, space="PSUM")
            )
            pt_pool = ctx.enter_context(
                tc.tile_pool(name="pt", bufs=1, space="PSUM")
            )

            if repeat > 1:
                ctx.enter_context(tc.For_i(0, repeat, 1))

            def stage0_m(m):
                xw = xw_pool.tile([128, HW], bf16, tag="xw")
                nc.sync.dma_start(out=xw, in_=x1s[m * 128:(m + 1) * 128, :])
                sq = sq_pool.tile([128, HW], bf16, tag="sq")
                sumsq = stats.tile([128, 1], f32, tag="sumsq")
                nc.scalar.activation(
                    out=sq, in_=xw, func=AF.Square, accum_out=sumsq
                )
                sq2 = sq_pool.tile([128, HW], bf16, tag="sq2")
                s1 = stats.tile([128, 1], f32, tag="s1")
                nc.scalar.activation(
                    out=sq2, in_=xw, func=AF.Copy, accum_out=s1
                )
                nrm = stats.tile([128, 1], f32, tag="nrm")
                nc.scalar.activation(out=nrm, in_=sumsq, func=AF.Sqrt)
                rn = stats.tile([128, 1], f32, tag="rn")
                nc.vector.reciprocal(out=rn, in_=nrm)
                ms = stats.tile([128, 1], f32, tag="ms")
                nc.vector.tensor_scalar(
                    out=ms, in0=s1, scalar1=1.0 / HW, scalar2=None, op0=OP.mult
                )
                # base = 1 - hw*mq^2, mq = mean of the normalized row
                mq = stats.tile([128, 1], f32, tag="mq")
                nc.vector.tensor_scalar(
                    out=mq, in0=ms, scalar1=rn, scalar2=None, op0=OP.mult
                )
                msq = stats.tile([128, 1], f32, tag="msq")
                nc.vector.tensor_scalar(
                    out=msq, in0=mq, scalar1=mq, scalar2=None, op0=OP.mult
                )
                nc.vector.tensor_scalar(
                    out=base_t[:, m:m + 1], in0=msq,
                    scalar1=-float(HW), scalar2=1.0,
                    op0=OP.mult, op1=OP.add,
                )
                dn = dn_pool.tile([128, HW], f32r, tag="dn")
                nc.vector.tensor_scalar(
                    out=dn, in0=xw, scalar1=ms, scalar2=rn,
                    op0=OP.subtract, op1=OP.mult,
                )
                nc.vector.tensor_copy(out=d_res[:, m, 0, :], in_=dn[:, 0:392])
                nc.vector.tensor_copy(out=d_res[:, m, 1, :], in_=dn[:, 392:784])
                for kt, (koff, klen) in enumerate(KT):
                    pt = pt_pool.tile([128, 128], f32r, tag="pt")
                    nc.tensor.transpose(
                        out=pt[:klen, :], in_=dn[:, koff:koff + klen],
                        identity=ident,
                    )
                    dst = dtT[:klen, m, kt, :]
                    if kt % 2 == 0:
                        nc.scalar.mul(out=dst, in_=pt[:klen, :], mul=SD)
                    else:
                        nc.vector.tensor_scalar(
                            out=dst, in0=pt[:klen, :], scalar1=SD, scalar2=None, op0=OP.mult
                        )

            def gram_class(n):
                xs = xs_pool.tile([128, RTN, HW], fp8, tag="xs")
                for rt in range(RTN):
                    nc.sync.dma_start(out=xs[:, rt, :], in_=x2d[n, rt, :, :])
                srow = row_pool.tile([1, HW], fp8, tag="srow")
                ssrow = row_pool.tile([1, HW], fp8, tag="ssrow")
                for qi, (qoff, qlen) in enumerate(QT):
                    pm = pm_pool.tile([1, 512], f32, tag="pm")
                    for r in range(RTN // 2):
                        nc.tensor.matmul(
                            pm[:1, :qlen],
                            lhsT=ones2[:, :, 0:1],
                            rhs=xs[:, 2 * r:2 * r + 2, qoff:qoff + qlen],
                            start=(r == 0), stop=(r == RTN // 2 - 1),
                            perf_mode=DRM,
                        )
                    qs = slice(qoff, qoff + qlen)
                    nc.scalar.mul(out=srow[:, qs], in_=pm[:1, :qlen], mul=1.0)
                    nc.scalar.mul(
                        out=ssrow[:, qs], in_=pm[:1, :qlen], mul=-1.0 / NR
                    )
                for mc, (mcoff, mclen) in enumerate(KT):
                    for qi, (qoff, qlen) in enumerate(QT):
                        psg = psg_pool.tile([128, 512], f32, tag="psg")
                        for r in range(RTN // 2):
                            nc.tensor.matmul(
                                psg[:mclen, :qlen],
                                lhsT=xs[:, 2 * r:2 * r + 2,
                                        mcoff:mcoff + mclen],
                                rhs=xs[:, 2 * r:2 * r + 2, qoff:qoff + qlen],
                                start=(r == 0), stop=False,
                                perf_mode=DRM,
                            )
                        has_diag = (mcoff < qoff + qlen
                                    and qoff < mcoff + mclen)
                        nc.tensor.matmul(
                            psg[:mclen, :qlen],
                            lhsT=ssrow[:1, mcoff:mcoff + mclen],
                            rhs=srow[:1, qoff:qoff + qlen],
                            start=False, stop=not has_diag,
                            skip_group_check=True,
                        )
                        if has_diag:
                            s_off = EYE_OFF - mcoff + qoff
                            nc.tensor.matmul(
                                psg[:mclen, :qlen],
                                lhsT=ai[:, :mclen],
                                rhs=eye[:, s_off:s_off + qlen],
                                start=False, stop=True,
                                skip_group_check=True,
                            )
                        dst = cov[:mclen, n, mc, qoff:qoff + qlen]
                        if qi == 0:
                            nc.vector.tensor_scalar(
                                out=dst, in0=psg[:mclen, :qlen],
                                scalar1=SC / (NR - 1), scalar2=None, op0=OP.mult,
                            )
                        else:
                            nc.scalar.mul(
                                out=dst, in_=psg[:mclen, :qlen],
                                mul=SC / (NR - 1),
                            )

            # ---- Phase A: stage-0 m-tiles with gram classes interleaved ----
            for i in range(MT):
                stage0_m(i)
                if 4 <= i < 4 + NW:
                    gram_class(i - 4)

            # ---- Phase B: sim = (D @ V) . D row-reduced ----
            for n in range(NW):
                for m in range(MT):
                    ps = ps_pool.tile([128, 2, 512], f32, tag="ps")
                    for qi, (qoff, qlen) in enumerate(QT):
                        for t in range(NKT8 // 2):
                            nc.tensor.matmul(
                                ps[:, qi, :qlen],
                                lhsT=dtT[:, m, 2 * t:2 * t + 2, :],
                                rhs=cov[:, n, 2 * t:2 * t + 2,
                                        qoff:qoff + qlen],
                                start=(t == 0), stop=(t == NKT8 // 2 - 1),
                                perf_mode=DRM,
                            )
                    scr = scr_pool.tile([128, 2, 392], bf16, tag="scr")
                    nc.vector.scalar_tensor_tensor(
                        out=scr,
                        in0=ps[:, :, :392],
                        scalar=1.0 / (SD * SC),
                        in1=d_res[:, m, :, :],
                        op0=OP.mult, op1=OP.mult,
                        accum_out=out_acc[:, m, n:n + 1],
                    )

            for m in range(MT):
                nc.vector.tensor_scalar(
                    out=out_acc[:, m, :], in0=out_acc[:, m, :],
                    scalar1=base_t[:, m:m + 1], scalar2=None, op0=OP.add,
                )
                nc.sync.dma_start(out=out[m], in_=out_acc[:, m, :])
            if debug:
                nc.sync.dma_start(out=dbg_dtT[:, :, :, :], in_=dtT)
                nc.sync.dma_start(out=dbg_cov[:, :, :, :], in_=cov)
                nc.sync.dma_start(out=dbg_base[:, :], in_=base_t)
                nc.sync.dma_start(out=dbg_dres[:, :, :, :], in_=d_res)

    nc.finalize()
    return nc


def get_program():
    key = "nc"
    if key not in _STATE:
        _STATE[key] = _build_program()
    return _STATE[key]


def make_in_maps(x1, x2):
    import ml_dtypes

    x1f = np.asarray(x1, dtype=np.float32).reshape(B * C, HW)
    x1b = np.ascontiguousarray(x1f).astype(ml_dtypes.bfloat16)
    x2f = np.asarray(x2, dtype=np.float32).reshape(NW, RTN, 128, HW)
    x2q = np.ascontiguousarray(x2f).astype(ml_dtypes.float8_e4m3)
    return [
        {"x1s": x1b[c * NI:(c + 1) * NI], "x2": x2q}
        for c in range(N_CORES)
    ]


def assemble_output(core_outs):
    # per-core (MT, 128, NW) -> (BSH, NW*C); concat over cores -> (B, NW*C)
    parts = [
        o.reshape(NI, NW).reshape(BSH, C, NW).transpose(0, 2, 1)
        .reshape(BSH, NW * C)
        for o in core_outs
    ]
    return np.ascontiguousarray(np.concatenate(parts, axis=0), dtype=np.float32)


def kernel(x1, x2):
    from concourse.bass_utils import run_bass_kernel_spmd

    nc = get_program()
    in_maps = make_in_maps(x1, x2)
    res = run_bass_kernel_spmd(nc, in_maps, list(range(N_CORES)))
    return assemble_output([res.results[i]["out"] for i in range(N_CORES)])
